# revision 27
# baseline (speedup 1.0000x reference)
"""MoE (top-1 routed) Trainium2 kernel.

Strategy: the reference computes every expert for every token and then
selects one expert per token with a one-hot gate.  Mathematically the
output for token n is expert_out[argmax_e logits[n, e], n], so we compute
the gating on host (bitwise-matching the reference's fp32 `x @ Wg + bg`
on CPU), group tokens by their selected expert, and run expert e's
pipeline for only its own tokens on NeuronCore e (expert-parallel, an
all-reduce-free gather).  This is 8x less device compute than the dense
reference formulation.

Device pipeline per core (C = padded token count, transposed layout with
features on partitions and tokens on the free dim):
    h^T[u, n]  = W1^T x^T          (PE, K=1024 accumulated in PSUM)
    sw         = (tanh(h/2) + 1) * h            # == 2*swish(h)
    z^T[v, n]  = (0.5*proj)^T sw   (PE)         # 0.5 folds the 2 above
    t2         = tanh(z/2)                      # == 2*sigmoid(z) - 1
    q          = exp((32/7) * t2)               # ONE exp per block
      -- the reference's gaussian basis times exp(32*xn^2) is
         g_j = exp(32*k_j*t2 + 32*k_j*(1-k_j)) = c_j * q^j  (k_j = j/7),
         so the basis numerator/denominator are degree-7 polynomials in
         q.  The per-element factor cancels in the normalization (the
         reference's +1e-6 in the denominator is a <=1.2e-6 relative
         perturbation, below fp32 matmul noise).
    powers q^2..q^7 via ACT square + DVE/GPSIMD multiplies (bf16)
    num        = sum_j (cv_j*c_j) q^j   (PE; cv = ctrl * scaling)
    den        = 1 + sum_j c_j q^j      (PE; the 1 via a ones tile)
    out^T[u,n] = (num + cv_0) * reciprocal(den)

tanh/exp/square share one ACT table set ("exp_and_others"), so the
scalar engine never pays the ~2.7us table switch.

num/den accumulate on the PE as plain 128-wide diagonal matmuls
(consecutive same-bank matmuls pipeline back-to-back at ~N/2.4GHz;
LDWEIGHTS hides under the stream).  The whole matmul path runs bf16
(hw rel err 6.3e-3 vs the 2e-2 budget, fp32 PSUM accumulation).

Performance structure (per core, C=1152 -> tiles [512, 512, 128]):
  - software-pipelined PE queue  h(i) -> nd(i-1) -> z(i) -> h(i+1) ...
    so the swish / power-chain elementwise latencies are always covered
    by matmul work and the PE HAM clock stays at 8/8 (2.4 GHz).
  - warmup matmuls (10xN512 + 45xN128 fillers) run during the input
    DMA window so the HAM is warm when real work starts.
  - x is ONE resident [P, 8, C] buffer loaded per-kc with 2.3KB
    contiguous lines, interleaved with w1 across the sync/scalar DMA
    queues (~100 GB/s each); gpsimd SWDGE carries small constants
    earliest-needed-first.  kc-outer h-matmuls consume the slices as
    they land.
  - the ragged remainder tile runs LAST (shortest finals tail).
"""

import os
from contextlib import ExitStack

import numpy as np

N_TOK, D_IN, U_DIM, E_EXP, B_BAS = 8192, 1024, 512, 8, 8
N_CORES = 8
P = 128
TNMAX = 512

PACK_MODE = os.environ.get("MOE_PACK", "plain")  # "plain" | "tile4"
SQ_ACT = int(os.environ.get("MOE_SQ_ACT", "2"))  # q2/q4 via ACT square (0-2)
N_PW_DVE = int(os.environ.get("MOE_PW_DVE", "3"))  # TT powers on DVE (rest GPSIMD)
G_BUFS = int(os.environ.get("MOE_GBUFS", "56"))
X_BUFS = int(os.environ.get("MOE_XBUFS", "3"))
PS_BUFS = int(os.environ.get("MOE_PS_BUFS", "8"))

_prog_cache = {}


def _tiling(C):
    """Full-width (512) token tiles, with any ragged remainder placed
    SECOND: N=512 matmuls are the cheapest per token, and sandwiching
    the low-duty remainder tile mid-stream keeps the PE busy enough
    that the HAM never re-throttles; the last tile (whose finals and
    output DMA form the kernel tail) is always a full tile."""
    full, rem = divmod(C, TNMAX)
    sizes = [TNMAX] * full
    if rem:
        sizes.append(rem)
    tiles = []
    t0 = 0
    for s in sizes:
        tiles.append((t0, s))
        t0 += s
    return tiles


def _knot_consts():
    # g_j = exp(32*k_j*t2 + 32*k_j*(1-k_j)) = c_j * q^j,  q = exp((32/7)*t2)
    ks = np.linspace(0.0, 1.0, B_BAS).astype(np.float64)
    cj = np.exp(32.0 * ks * (1.0 - ks))  # c_0 = c_7 = 1
    return ks, cj


def build_program(C, mm_mode, b1_zero):
    """Build + compile the SPMD single-core program for capacity C."""
    import concourse.tile as tile
    from concourse import bacc, mybir

    f32 = mybir.dt.float32
    f32r = mybir.dt.float32r
    bf16 = mybir.dt.bfloat16
    add = mybir.AluOpType.add
    mult = mybir.AluOpType.mult
    Tanh = mybir.ActivationFunctionType.Tanh
    Exp = mybir.ActivationFunctionType.Exp
    Square = mybir.ActivationFunctionType.Square

    mm_dt = bf16
    g_dt = bf16

    assert C % P == 0
    tiles = _tiling(C)

    _, cj = _knot_consts()
    QS = 32.0 / 7.0  # exp scale

    nc = bacc.Bacc("TRN2", target_bir_lowering=False, debug=False,
                   num_devices=1)

    # all inputs partition-major with long contiguous per-partition lines
    NT = len(tiles)
    xT = nc.dram_tensor("xT", [NT, P, 8, TNMAX], mm_dt,
                        kind="ExternalInput").ap()
    w1 = nc.dram_tensor("w1", [P, 8, U_DIM], mm_dt,
                        kind="ExternalInput").ap()
    p5 = nc.dram_tensor("p5", [P, 4, U_DIM], mm_dt,
                        kind="ExternalInput").ap()
    auxn = nc.dram_tensor("auxn", [P, 28, P], g_dt,
                          kind="ExternalInput").ap()
    auxd = nc.dram_tensor("auxd", [P, 8, P], g_dt,
                          kind="ExternalInput").ap()
    a0h = nc.dram_tensor("a0h", [P, 4], f32, kind="ExternalInput").ap()
    onesd = nc.dram_tensor("onesd", [P, TNMAX], g_dt,
                           kind="ExternalInput").ap()
    b1h = nc.dram_tensor("b1h", [P, 4], f32, kind="ExternalInput").ap()
    outT = nc.dram_tensor("outT", [U_DIM, C], f32, kind="ExternalOutput").ap()

    outT_r = outT.rearrange("(vc p) c -> p vc c", p=P)

    with tile.TileContext(nc) as tc, ExitStack() as ctx:
        cpool = ctx.enter_context(tc.tile_pool(name="consts", bufs=1))
        xpool = ctx.enter_context(tc.tile_pool(name="x", bufs=X_BUFS))
        pspool = ctx.enter_context(tc.tile_pool(name="ps", bufs=PS_BUFS,
                                                space="PSUM"))
        epool = ctx.enter_context(tc.tile_pool(name="elem", bufs=4))
        swpool = ctx.enter_context(tc.tile_pool(name="sw", bufs=6))
        gpool = ctx.enter_context(tc.tile_pool(name="g", bufs=G_BUFS))
        mpool = ctx.enter_context(tc.tile_pool(name="m", bufs=4))
        opool = ctx.enter_context(tc.tile_pool(name="o", bufs=2))

        # startup-critical loads interleaved across the two hardware DMA
        # queues (sync/scalar, ~100 GB/s each); x is ONE resident buffer
        # [P, 8, C] loaded per-kc (2.3KB contiguous lines) and sliced by
        # every tile, so only tile 0 ever waits on x.  gpsimd SWDGE
        # carries the small late-needed constants, earliest-needed first.
        # tile-major x: tile 0's block loads FIRST (4KB contiguous
        # lines), interleaved with w1 quarters, so tile-0's h is fed by
        # ~19us instead of waiting on full-C rows (~26us); later tiles'
        # blocks follow and land before their h stages need them.
        w1sb = cpool.tile([P, 8, U_DIM], mm_dt, tag="w1")
        xq = []
        for ti, (t0, TN) in enumerate(tiles):
            xa = xpool.tile([P, 8, TNMAX], mm_dt, tag="xa", name=f"xa{t0}")
            if ti == 0:
                nc.sync.dma_start(w1sb[:, 0:2, :], w1[:, 0:2, :])
                nc.scalar.dma_start(w1sb[:, 2:4, :], w1[:, 2:4, :])
            nc.sync.dma_start(xa[:, 0:4, :TN], xT[ti, :, 0:4, :TN])
            nc.scalar.dma_start(xa[:, 4:8, :TN], xT[ti, :, 4:8, :TN])
            if ti == 0:
                nc.sync.dma_start(w1sb[:, 4:6, :], w1[:, 4:6, :])
                nc.scalar.dma_start(w1sb[:, 6:8, :], w1[:, 6:8, :])
            xq.append(xa)
        w1k = [w1sb[:, kc, :] for kc in range(8)]
        ones = cpool.tile([P, TNMAX], g_dt, tag="ones")
        nc.gpsimd.dma_start(ones[:], onesd[:])
        auxdsb = cpool.tile([P, 8, P], g_dt, tag="auxd")
        nc.gpsimd.dma_start(auxdsb[:], auxd[:])
        p5sb = cpool.tile([P, 4, U_DIM], mm_dt, tag="p5")
        nc.gpsimd.dma_start(p5sb[:], p5[:])
        puc = [p5sb[:, uc, :] for uc in range(4)]
        auxnsb = cpool.tile([P, 28, P], g_dt, tag="auxn")
        nc.gpsimd.dma_start(auxnsb[:], auxn[:])
        a0sb = cpool.tile([P, 4], f32, tag="a0h")
        nc.gpsimd.dma_start(a0sb[:], a0h[:])
        if not b1_zero:
            b1sb = cpool.tile([P, 4], f32, tag="b1h")
            nc.gpsimd.dma_start(b1sb[:], b1h[:])

        def stage_h_mm(ti):
            """h matmuls (kc-outer: consumes w1/x slices as DMA delivers
            them)."""
            t0, TN = tiles[ti]
            hps = [pspool.tile([P, TNMAX], f32, tag="ps", name=f"hps{uc}")
                   for uc in range(4)]
            xa = xq[ti]
            for kc in range(8):
                for uc in range(4):
                    nc.tensor.matmul(
                        hps[uc][:, :TN],
                        lhsT=w1k[kc][:, uc * P:(uc + 1) * P],
                        rhs=xa[:, kc, :TN],
                        start=(kc == 0), stop=(kc == 7),
                    )
            return hps

        def stage_h_act(ti, hps):
            """tanh + swish for tile ti; emitted AFTER stage_nd(ti-1) so
            recip/final(i-1) precede sw(i) in the DVE FIFO (frees the
            nd PSUM banks before z(i) needs them)."""
            t0, TN = tiles[ti]
            sws = []
            for uc in range(4):
                th = epool.tile([P, TNMAX], f32, tag="th")
                if b1_zero:
                    nc.scalar.activation(th[:, :TN], hps[uc][:, :TN], Tanh,
                                         scale=0.5)
                else:
                    nc.scalar.activation(th[:, :TN], hps[uc][:, :TN], Tanh,
                                         scale=0.5, bias=b1sb[:, uc:uc + 1])
                sw = swpool.tile([P, TNMAX], mm_dt, tag="sw")
                if b1_zero:
                    # sw = (th + 1) * h  == 2*swish(h)
                    nc.vector.scalar_tensor_tensor(
                        sw[:, :TN], th[:, :TN], 1.0, hps[uc][:, :TN],
                        op0=add, op1=mult)
                else:
                    y = epool.tile([P, TNMAX], f32, tag="y")
                    nc.vector.tensor_scalar(
                        y[:, :TN], hps[uc][:, :TN], b1sb[:, uc:uc + 1],
                        None, op0=add)
                    nc.vector.scalar_tensor_tensor(
                        sw[:, :TN], th[:, :TN], 1.0, y[:, :TN],
                        op0=add, op1=mult)
                sws.append(sw)
            return sws

        def stage_z(ti, sws):
            """z matmuls + tanh + exp + bf16 power tiles."""
            t0, TN = tiles[ti]
            pw = []  # pw[vc] = [None, q, q2, ..., q7]
            for vc in range(4):
                zps = pspool.tile([P, TNMAX], f32, tag="ps", name="zps")
                for uc in range(4):
                    nc.tensor.matmul(
                        zps[:, :TN],
                        lhsT=puc[uc][:, vc * P:(vc + 1) * P],
                        rhs=sws[uc][:, :TN],
                        start=(uc == 0), stop=(uc == 3),
                    )
                t2 = epool.tile([P, TNMAX], f32, tag="t2")
                nc.scalar.activation(t2[:, :TN], zps[:, :TN], Tanh, scale=0.5)

                q = [None] * 8
                for j in (1, 2, 3, 4, 5, 6, 7):
                    q[j] = gpool.tile([P, TNMAX], g_dt, tag="g",
                                      name=f"q{j}_{vc}")
                nc.scalar.activation(q[1][:, :TN], t2[:, :TN], Exp, scale=QS)
                # squares on ACT (knob), remaining powers as TT products
                tt_plan = []
                if SQ_ACT >= 1:
                    nc.scalar.activation(q[2][:, :TN], q[1][:, :TN], Square)
                else:
                    tt_plan.append((2, 1, 1))
                if SQ_ACT >= 2:
                    nc.scalar.activation(q[4][:, :TN], q[2][:, :TN], Square)
                else:
                    tt_plan.append((4, 2, 2))
                tt_plan += [(3, 1, 2), (5, 1, 4), (6, 2, 4), (7, 3, 4)]
                for idx, (jo, ja, jb) in enumerate(tt_plan):
                    eng = nc.vector if idx < N_PW_DVE else nc.gpsimd
                    eng.tensor_tensor(q[jo][:, :TN], q[ja][:, :TN],
                                      q[jb][:, :TN], mult)
                pw.append(q)
            return pw

        def stage_nd(ti, pw):
            """num/den diag-matmul chains + recip/final + out DMA."""
            t0, TN = tiles[ti]
            outb = opool.tile([P, 4, TNMAX], f32, tag="outb")
            for vc in range(4):
                q = pw[vc]
                nps = pspool.tile([P, TNMAX], f32, tag="ps", name="nps")
                dps = pspool.tile([P, TNMAX], f32, tag="ps", name="dps")
                # consecutive same-bank matmuls pipeline back-to-back;
                # alternating banks exposes the PE drain on every MM.
                # den first so recip overlaps the num chain.
                nc.tensor.matmul(
                    dps[:, :TN], lhsT=auxdsb[:, 0, :], rhs=ones[:, :TN],
                    start=True, stop=False)
                for j in range(1, 8):
                    nc.tensor.matmul(
                        dps[:, :TN],
                        lhsT=auxdsb[:, j, :],
                        rhs=q[j][:, :TN],
                        start=False, stop=(j == 7))
                for j in range(1, 8):
                    nc.tensor.matmul(
                        nps[:, :TN],
                        lhsT=auxnsb[:, vc * 7 + (j - 1), :],
                        rhs=q[j][:, :TN],
                        start=(j == 1), stop=(j == 7))
                r = mpool.tile([P, TNMAX], f32, tag="r", name=f"r{vc}")
                nc.vector.reciprocal_approx_fast(r[:, :TN], dps[:, :TN])
                nc.vector.scalar_tensor_tensor(
                    outb[:, vc, :TN], nps[:, :TN], a0sb[:, vc:vc + 1],
                    r[:, :TN], op0=add, op1=mult)
                oeng = nc.sync if vc % 2 == 0 else nc.scalar
                oeng.dma_start(outT_r[:, vc, t0:t0 + TN],
                               outb[:, vc, :TN])

        # PE warmup: garbage matmuls during the x-load window get the
        # HAM past its 3.4us SHORT window so the first real matmuls run
        # at 2.4 GHz instead of 1.2
        wsrc = cpool.tile([P, TNMAX], mm_dt, tag="warm")
        nc.gpsimd.memset(wsrc[:], 0.0)
        wps = pspool.tile([P, TNMAX], f32, tag="ps", name="warm")
        for _ in range(10):
            nc.tensor.matmul(wps[:], lhsT=wsrc[:, :P], rhs=wsrc[:],
                             start=True, stop=True)
        # fine-grained fillers: keep the PE busy until the x/w1 DMA
        # lands; each wastes at most ~54ns once real work is ready
        for _ in range(24):
            nc.tensor.matmul(wps[:, :P], lhsT=wsrc[:, :P], rhs=wsrc[:, :P],
                             start=True, stop=True)

        # software pipeline on the PE queue:
        #   h(i) -> nd(i-1) -> z(i) -> h(i+1) -> nd(i) -> ...
        # nd(i-1) covers the swish latency between h(i) and z(i);
        # h(i+1) covers the power-chain latency between z(i) and nd(i).
        prev = None
        for ti in range(len(tiles)):
            sws = stage_h_act(ti, stage_h_mm(ti))
            if prev is not None:
                stage_nd(ti - 1, prev)
            prev = stage_z(ti, sws)
        stage_nd(len(tiles) - 1, prev)

    nc.compile()
    return nc, tiles


def _get_program(C, mm_mode, b1_zero):
    key = (C, mm_mode, b1_zero, PACK_MODE, SQ_ACT, N_PW_DVE, G_BUFS, X_BUFS,
           PS_BUFS)
    if key not in _prog_cache:
        _prog_cache[key] = build_program(C, mm_mode, b1_zero)
    return _prog_cache[key]


def _route_on_host(x, Wg, bg):
    """Expert assignment, bitwise-matching the reference's fp32 CPU math."""
    import jax
    import jax.numpy as jnp

    cpu = jax.devices("cpu")[0]
    with jax.default_device(cpu):
        logits = jnp.asarray(x) @ jnp.asarray(Wg) + jnp.asarray(bg)
        eid = np.asarray(jnp.argmax(logits, axis=-1))
    return eid


def make_in_maps(x, W1, b1, proj, ctrl, scaling, Wg, bg, mm_mode="f32r"):
    import ml_dtypes

    x = np.asarray(x, dtype=np.float32)
    eid = _route_on_host(x, Wg, bg)
    order = np.argsort(eid, kind="stable")
    counts = np.bincount(eid, minlength=E_EXP)
    starts = np.zeros(E_EXP + 1, dtype=np.int64)
    starts[1:] = np.cumsum(counts)
    C = int(max(counts.max(), 1))
    C = ((C + P - 1) // P) * P

    _, cj = _knot_consts()

    cvf = (np.asarray(ctrl, np.float32)
           * np.asarray(scaling, np.float32)[:, None, :])  # [E, B, U]
    proj5 = 0.5 * np.asarray(proj, np.float32)
    b1f = np.asarray(b1, np.float32)
    b1_zero = not np.any(b1f)

    g_np = ml_dtypes.bfloat16
    ar = np.arange(P)
    tiling = _tiling(C)
    NT = len(tiling)

    in_maps = []
    for e in range(E_EXP):
        idx = order[starts[e]:starts[e + 1]]
        # [NT, P, 8, TNMAX]: xT[t, p, kc, n] = x[token t0+n, kc*128+p]
        xTf = np.zeros((D_IN, C), dtype=np.float32)
        if len(idx):
            xTf[:, :len(idx)] = x[idx].T
        xT = np.zeros((len(tiling), P, 8, TNMAX), dtype=g_np)
        for ti, (t0, TN) in enumerate(tiling):
            xT[ti, :, :, :TN] = (
                xTf[:, t0:t0 + TN].reshape(8, P, TN).transpose(1, 0, 2))
        w1d = np.ascontiguousarray(
            np.asarray(W1[e], np.float32).reshape(8, P, U_DIM)
            .transpose(1, 0, 2)).astype(g_np)
        p5d = np.ascontiguousarray(
            proj5[e].reshape(4, P, U_DIM).transpose(1, 0, 2)).astype(g_np)
        b1h = np.ascontiguousarray(
            (0.5 * b1f[e]).reshape(4, P).T).astype(np.float32)
        auxn = np.zeros((P, 28, P), dtype=np.float32)
        auxd = np.zeros((P, 8, P), dtype=np.float32)
        for vc in range(4):
            for j in range(1, 8):
                auxn[ar, vc * 7 + (j - 1), ar] = \
                    cvf[e][j, vc * P:(vc + 1) * P] * cj[j]
        for j in range(8):
            auxd[ar, j, ar] = 1.0 if j == 0 else cj[j]
        a0 = np.zeros((P, 4), dtype=np.float32)
        for vc in range(4):
            a0[:, vc] = cvf[e][0, vc * P:(vc + 1) * P]
        in_maps.append({
            "xT": xT,
            "w1": w1d,
            "p5": p5d,
            "auxn": auxn.astype(g_np),
            "auxd": auxd.astype(g_np),
            "a0h": a0,
            "b1h": b1h,
            "onesd": np.ones((P, TNMAX), dtype=g_np),
        })
    return in_maps, order, starts, counts, C, b1_zero


def kernel(x, W1, b1, proj, ctrl, scaling, Wg, bg):
    from concourse.bass_utils import run_bass_kernel_spmd

    in_maps, order, starts, counts, C, b1_zero = make_in_maps(
        x, W1, b1, proj, ctrl, scaling, Wg, bg)
    nc, _ = _get_program(C, "f32r", b1_zero)

    res = run_bass_kernel_spmd(nc, in_maps, list(range(N_CORES)))

    out = np.empty((N_TOK, U_DIM), dtype=np.float32)
    for e in range(E_EXP):
        cnt = int(counts[e])
        if cnt:
            out[order[starts[e]:starts[e + 1]]] = \
                res.results[e]["outT"][:, :cnt].T
    return out


MM_MODE = "f32r"  # kept for test.py compatibility


# revision 30
# speedup vs baseline: 1.0387x; 1.0387x over previous
"""MoE (top-1 routed) Trainium2 kernel.

Strategy: the reference computes every expert for every token and then
selects one expert per token with a one-hot gate.  Mathematically the
output for token n is expert_out[argmax_e logits[n, e], n], so we compute
the gating on host (bitwise-matching the reference's fp32 `x @ Wg + bg`
on CPU), group tokens by their selected expert, and run expert e's
pipeline for only its own tokens on NeuronCore e (expert-parallel, an
all-reduce-free gather).  This is 8x less device compute than the dense
reference formulation.

Device pipeline per core (C = padded token count, transposed layout with
features on partitions and tokens on the free dim):
    h^T[u, n]  = W1^T x^T          (PE, K=1024 accumulated in PSUM)
    sw         = (tanh(h/2) + 1) * h            # == 2*swish(h)
    z^T[v, n]  = (0.5*proj)^T sw   (PE)         # 0.5 folds the 2 above
    t2         = tanh(z/2)                      # == 2*sigmoid(z) - 1
    q          = exp((32/7) * t2)               # ONE exp per block
      -- the reference's gaussian basis times exp(32*xn^2) is
         g_j = exp(32*k_j*t2 + 32*k_j*(1-k_j)) = c_j * q^j  (k_j = j/7),
         so the basis numerator/denominator are degree-7 polynomials in
         q.  The per-element factor cancels in the normalization (the
         reference's +1e-6 in the denominator is a <=1.2e-6 relative
         perturbation, below fp32 matmul noise).
    powers q^2..q^7 via ACT square + DVE/GPSIMD multiplies (bf16)
    num        = sum_j (cv_j*c_j) q^j   (PE; cv = ctrl * scaling)
    den        = 1 + sum_j c_j q^j      (PE; the 1 via a ones tile)
    out^T[u,n] = (num + cv_0) * reciprocal(den)

tanh/exp/square share one ACT table set ("exp_and_others"), so the
scalar engine never pays the ~2.7us table switch.

num/den accumulate on the PE as plain 128-wide diagonal matmuls
(consecutive same-bank matmuls pipeline back-to-back at ~N/2.4GHz;
LDWEIGHTS hides under the stream).  The whole matmul path runs bf16
(hw rel err 6.3e-3 vs the 2e-2 budget, fp32 PSUM accumulation).

Performance structure (per core, C=1152 -> tiles [512, 512, 128]):
  - software-pipelined PE queue  h(i) -> nd(i-1) -> z(i) -> h(i+1) ...
    so the swish / power-chain elementwise latencies are always covered
    by matmul work and the PE HAM clock stays at 8/8 (2.4 GHz).
  - warmup matmuls (10xN512 + 45xN128 fillers) run during the input
    DMA window so the HAM is warm when real work starts.
  - x is ONE resident [P, 8, C] buffer loaded per-kc with 2.3KB
    contiguous lines, interleaved with w1 across the sync/scalar DMA
    queues (~100 GB/s each); gpsimd SWDGE carries small constants
    earliest-needed-first.  kc-outer h-matmuls consume the slices as
    they land.
  - the ragged remainder tile runs LAST (shortest finals tail).
"""

import os
from contextlib import ExitStack

import numpy as np

N_TOK, D_IN, U_DIM, E_EXP, B_BAS = 8192, 1024, 512, 8, 8
N_CORES = 8
P = 128
TNMAX = 512

PACK_MODE = os.environ.get("MOE_PACK", "plain")  # "plain" | "tile4"
SQ_ACT = int(os.environ.get("MOE_SQ_ACT", "2"))  # q2/q4 via ACT square (0-2)
N_PW_DVE = int(os.environ.get("MOE_PW_DVE", "3"))  # TT powers on DVE (rest GPSIMD)
G_BUFS = int(os.environ.get("MOE_GBUFS", "56"))
X_BUFS = int(os.environ.get("MOE_XBUFS", "3"))
PS_BUFS = int(os.environ.get("MOE_PS_BUFS", "8"))

_prog_cache = {}


def _tiling(C):
    """Full-width (512) token tiles, with any ragged remainder placed
    SECOND: N=512 matmuls are the cheapest per token, and sandwiching
    the low-duty remainder tile mid-stream keeps the PE busy enough
    that the HAM never re-throttles; the last tile (whose finals and
    output DMA form the kernel tail) is always a full tile."""
    full, rem = divmod(C, TNMAX)
    sizes = [TNMAX] * full
    if rem:
        sizes.append(rem)
    tiles = []
    t0 = 0
    for s in sizes:
        tiles.append((t0, s))
        t0 += s
    return tiles


def _knot_consts():
    # g_j = exp(32*k_j*t2 + 32*k_j*(1-k_j)) = c_j * q^j,  q = exp((32/7)*t2)
    ks = np.linspace(0.0, 1.0, B_BAS).astype(np.float64)
    cj = np.exp(32.0 * ks * (1.0 - ks))  # c_0 = c_7 = 1
    return ks, cj


def build_program(C, mm_mode, b1_zero):
    """Build + compile the SPMD single-core program for capacity C."""
    import concourse.tile as tile
    from concourse import bacc, mybir

    f32 = mybir.dt.float32
    f32r = mybir.dt.float32r
    bf16 = mybir.dt.bfloat16
    add = mybir.AluOpType.add
    mult = mybir.AluOpType.mult
    Tanh = mybir.ActivationFunctionType.Tanh
    Exp = mybir.ActivationFunctionType.Exp
    Square = mybir.ActivationFunctionType.Square

    mm_dt = bf16
    g_dt = bf16

    assert C % P == 0
    tiles = _tiling(C)

    _, cj = _knot_consts()
    QS = 32.0 / 7.0  # exp scale

    nc = bacc.Bacc("TRN2", target_bir_lowering=False, debug=False,
                   num_devices=1)

    # all inputs partition-major with long contiguous per-partition lines
    NT = len(tiles)
    xT = nc.dram_tensor("xT", [NT, P, 8, TNMAX], mm_dt,
                        kind="ExternalInput").ap()
    w1 = nc.dram_tensor("w1", [P, 8, U_DIM], mm_dt,
                        kind="ExternalInput").ap()
    p5 = nc.dram_tensor("p5", [P, 4, U_DIM], mm_dt,
                        kind="ExternalInput").ap()
    auxn = nc.dram_tensor("auxn", [P, 28, P], g_dt,
                          kind="ExternalInput").ap()
    auxd = nc.dram_tensor("auxd", [P, 8, P], g_dt,
                          kind="ExternalInput").ap()
    a0h = nc.dram_tensor("a0h", [P, 4], f32, kind="ExternalInput").ap()
    onesd = nc.dram_tensor("onesd", [P, TNMAX], g_dt,
                           kind="ExternalInput").ap()
    b1h = nc.dram_tensor("b1h", [P, 4], f32, kind="ExternalInput").ap()
    outT = nc.dram_tensor("outT", [U_DIM, C], f32, kind="ExternalOutput").ap()

    outT_r = outT.rearrange("(vc p) c -> p vc c", p=P)

    with tile.TileContext(nc) as tc, ExitStack() as ctx:
        cpool = ctx.enter_context(tc.tile_pool(name="consts", bufs=1))
        xpool = ctx.enter_context(tc.tile_pool(name="x", bufs=X_BUFS))
        pspool = ctx.enter_context(tc.tile_pool(name="ps", bufs=PS_BUFS,
                                                space="PSUM"))
        epool = ctx.enter_context(tc.tile_pool(name="elem", bufs=4))
        swpool = ctx.enter_context(tc.tile_pool(name="sw", bufs=6))
        gpool = ctx.enter_context(tc.tile_pool(name="g", bufs=G_BUFS))
        mpool = ctx.enter_context(tc.tile_pool(name="m", bufs=4))
        opool = ctx.enter_context(tc.tile_pool(name="o", bufs=2))

        # startup-critical loads interleaved across the two hardware DMA
        # queues (sync/scalar, ~100 GB/s each); x is ONE resident buffer
        # [P, 8, C] loaded per-kc (2.3KB contiguous lines) and sliced by
        # every tile, so only tile 0 ever waits on x.  gpsimd SWDGE
        # carries the small late-needed constants, earliest-needed first.
        # tile-major x: tile 0's block loads FIRST (4KB contiguous
        # lines), interleaved with w1 quarters, so tile-0's h is fed by
        # ~19us instead of waiting on full-C rows (~26us); later tiles'
        # blocks follow and land before their h stages need them.
        w1sb = cpool.tile([P, 8, U_DIM], mm_dt, tag="w1")
        xq = []
        for ti, (t0, TN) in enumerate(tiles):
            xa = xpool.tile([P, 8, TNMAX], mm_dt, tag="xa", name=f"xa{t0}")
            if ti == 0:
                nc.sync.dma_start(w1sb[:, 0:2, :], w1[:, 0:2, :])
                nc.scalar.dma_start(w1sb[:, 2:4, :], w1[:, 2:4, :])
            nc.sync.dma_start(xa[:, 0:4, :TN], xT[ti, :, 0:4, :TN])
            nc.scalar.dma_start(xa[:, 4:8, :TN], xT[ti, :, 4:8, :TN])
            if ti == 0:
                nc.sync.dma_start(w1sb[:, 4:6, :], w1[:, 4:6, :])
                nc.scalar.dma_start(w1sb[:, 6:8, :], w1[:, 6:8, :])
            xq.append(xa)
        w1k = [w1sb[:, kc, :] for kc in range(8)]
        ones = cpool.tile([P, TNMAX], g_dt, tag="ones")
        nc.gpsimd.dma_start(ones[:], onesd[:])
        auxdsb = cpool.tile([P, 8, P], g_dt, tag="auxd")
        nc.gpsimd.dma_start(auxdsb[:], auxd[:])
        p5sb = cpool.tile([P, 4, U_DIM], mm_dt, tag="p5")
        nc.gpsimd.dma_start(p5sb[:], p5[:])
        puc = [p5sb[:, uc, :] for uc in range(4)]
        auxnsb = cpool.tile([P, 28, P], g_dt, tag="auxn")
        nc.gpsimd.dma_start(auxnsb[:], auxn[:])
        a0sb = cpool.tile([P, 4], f32, tag="a0h")
        nc.gpsimd.dma_start(a0sb[:], a0h[:])
        if not b1_zero:
            b1sb = cpool.tile([P, 4], f32, tag="b1h")
            nc.gpsimd.dma_start(b1sb[:], b1h[:])

        def stage_h_mm(ti):
            """h matmuls (kc-outer: consumes w1/x slices as DMA delivers
            them)."""
            t0, TN = tiles[ti]
            hps = [pspool.tile([P, TNMAX], f32, tag="ps", name=f"hps{uc}")
                   for uc in range(4)]
            xa = xq[ti]
            for kc in range(8):
                for uc in range(4):
                    nc.tensor.matmul(
                        hps[uc][:, :TN],
                        lhsT=w1k[kc][:, uc * P:(uc + 1) * P],
                        rhs=xa[:, kc, :TN],
                        start=(kc == 0), stop=(kc == 7),
                    )
            return hps

        def stage_h_act(ti, hps):
            """tanh + swish for tile ti; emitted AFTER stage_nd(ti-1) so
            recip/final(i-1) precede sw(i) in the DVE FIFO (frees the
            nd PSUM banks before z(i) needs them)."""
            t0, TN = tiles[ti]
            sws = []
            for uc in range(4):
                th = epool.tile([P, TNMAX], f32, tag="th")
                if b1_zero:
                    nc.scalar.activation(th[:, :TN], hps[uc][:, :TN], Tanh,
                                         scale=0.5)
                else:
                    nc.scalar.activation(th[:, :TN], hps[uc][:, :TN], Tanh,
                                         scale=0.5, bias=b1sb[:, uc:uc + 1])
                sw = swpool.tile([P, TNMAX], mm_dt, tag="sw")
                if b1_zero:
                    # sw = (th + 1) * h  == 2*swish(h)
                    nc.vector.scalar_tensor_tensor(
                        sw[:, :TN], th[:, :TN], 1.0, hps[uc][:, :TN],
                        op0=add, op1=mult)
                else:
                    y = epool.tile([P, TNMAX], f32, tag="y")
                    nc.vector.tensor_scalar(
                        y[:, :TN], hps[uc][:, :TN], b1sb[:, uc:uc + 1],
                        None, op0=add)
                    nc.vector.scalar_tensor_tensor(
                        sw[:, :TN], th[:, :TN], 1.0, y[:, :TN],
                        op0=add, op1=mult)
                sws.append(sw)
            return sws

        def stage_z(ti, sws):
            """z matmuls + tanh + exp + bf16 power tiles."""
            t0, TN = tiles[ti]
            pw = []  # pw[vc] = [None, q, q2, ..., q7]
            for vc in range(4):
                zps = pspool.tile([P, TNMAX], f32, tag="ps", name="zps")
                for uc in range(4):
                    nc.tensor.matmul(
                        zps[:, :TN],
                        lhsT=puc[uc][:, vc * P:(vc + 1) * P],
                        rhs=sws[uc][:, :TN],
                        start=(uc == 0), stop=(uc == 3),
                    )
                t2 = epool.tile([P, TNMAX], f32, tag="t2")
                nc.scalar.activation(t2[:, :TN], zps[:, :TN], Tanh, scale=0.5)

                q = [None] * 8
                for j in (1, 2, 3, 4, 5, 6, 7):
                    q[j] = gpool.tile([P, TNMAX], g_dt, tag="g",
                                      name=f"q{j}_{vc}")
                nc.scalar.activation(q[1][:, :TN], t2[:, :TN], Exp, scale=QS)
                # squares on ACT (knob), remaining powers as TT products
                tt_plan = []
                if SQ_ACT >= 1:
                    nc.scalar.activation(q[2][:, :TN], q[1][:, :TN], Square)
                else:
                    tt_plan.append((2, 1, 1))
                if SQ_ACT >= 2:
                    nc.scalar.activation(q[4][:, :TN], q[2][:, :TN], Square)
                else:
                    tt_plan.append((4, 2, 2))
                tt_plan += [(3, 1, 2), (5, 1, 4), (6, 2, 4), (7, 3, 4)]
                for idx, (jo, ja, jb) in enumerate(tt_plan):
                    eng = nc.vector if idx < N_PW_DVE else nc.gpsimd
                    eng.tensor_tensor(q[jo][:, :TN], q[ja][:, :TN],
                                      q[jb][:, :TN], mult)
                pw.append(q)
            return pw

        def stage_nd(ti, pw):
            """num/den diag-matmul chains + recip/final + out DMA."""
            t0, TN = tiles[ti]
            outb = opool.tile([P, 4, TNMAX], f32, tag="outb")
            for vc in range(4):
                q = pw[vc]
                nps = pspool.tile([P, TNMAX], f32, tag="ps", name="nps")
                dps = pspool.tile([P, TNMAX], f32, tag="ps", name="dps")
                # consecutive same-bank matmuls pipeline back-to-back;
                # alternating banks exposes the PE drain on every MM.
                # den first so recip overlaps the num chain.
                nc.tensor.matmul(
                    dps[:, :TN], lhsT=auxdsb[:, 0, :], rhs=ones[:, :TN],
                    start=True, stop=False)
                for j in range(1, 8):
                    nc.tensor.matmul(
                        dps[:, :TN],
                        lhsT=auxdsb[:, j, :],
                        rhs=q[j][:, :TN],
                        start=False, stop=(j == 7))
                for j in range(1, 8):
                    nc.tensor.matmul(
                        nps[:, :TN],
                        lhsT=auxnsb[:, vc * 7 + (j - 1), :],
                        rhs=q[j][:, :TN],
                        start=(j == 1), stop=(j == 7))
                r = mpool.tile([P, TNMAX], f32, tag="r", name=f"r{vc}")
                nc.vector.reciprocal_approx_fast(r[:, :TN], dps[:, :TN])
                nc.vector.scalar_tensor_tensor(
                    outb[:, vc, :TN], nps[:, :TN], a0sb[:, vc:vc + 1],
                    r[:, :TN], op0=add, op1=mult)
                nc.sync.dma_start(outT_r[:, vc, t0:t0 + TN],
                                  outb[:, vc, :TN])

        # PE warmup: garbage matmuls during the x-load window get the
        # HAM past its 3.4us SHORT window so the first real matmuls run
        # at 2.4 GHz instead of 1.2
        wsrc = cpool.tile([P, TNMAX], mm_dt, tag="warm")
        nc.gpsimd.memset(wsrc[:], 0.0)
        wps = pspool.tile([P, TNMAX], f32, tag="ps", name="warm")
        for _ in range(10):
            nc.tensor.matmul(wps[:], lhsT=wsrc[:, :P], rhs=wsrc[:],
                             start=True, stop=True)
        # fine-grained fillers: keep the PE busy until the x/w1 DMA
        # lands; each wastes at most ~54ns once real work is ready
        for _ in range(24):
            nc.tensor.matmul(wps[:, :P], lhsT=wsrc[:, :P], rhs=wsrc[:, :P],
                             start=True, stop=True)

        # software pipeline on the PE queue:
        #   h(i) -> nd(i-1) -> z(i) -> h(i+1) -> nd(i) -> ...
        # nd(i-1) covers the swish latency between h(i) and z(i);
        # h(i+1) covers the power-chain latency between z(i) and nd(i).
        prev = None
        for ti in range(len(tiles)):
            sws = stage_h_act(ti, stage_h_mm(ti))
            if prev is not None:
                stage_nd(ti - 1, prev)
            prev = stage_z(ti, sws)
        stage_nd(len(tiles) - 1, prev)

    nc.compile()
    return nc, tiles


def _get_program(C, mm_mode, b1_zero):
    key = (C, mm_mode, b1_zero, PACK_MODE, SQ_ACT, N_PW_DVE, G_BUFS, X_BUFS,
           PS_BUFS)
    if key not in _prog_cache:
        _prog_cache[key] = build_program(C, mm_mode, b1_zero)
    return _prog_cache[key]


def _route_on_host(x, Wg, bg):
    """Expert assignment, bitwise-matching the reference's fp32 CPU math."""
    import jax
    import jax.numpy as jnp

    cpu = jax.devices("cpu")[0]
    with jax.default_device(cpu):
        logits = jnp.asarray(x) @ jnp.asarray(Wg) + jnp.asarray(bg)
        eid = np.asarray(jnp.argmax(logits, axis=-1))
    return eid


def make_in_maps(x, W1, b1, proj, ctrl, scaling, Wg, bg, mm_mode="f32r"):
    import ml_dtypes

    x = np.asarray(x, dtype=np.float32)
    eid = _route_on_host(x, Wg, bg)
    order = np.argsort(eid, kind="stable")
    counts = np.bincount(eid, minlength=E_EXP)
    starts = np.zeros(E_EXP + 1, dtype=np.int64)
    starts[1:] = np.cumsum(counts)
    C = int(max(counts.max(), 1))
    C = ((C + P - 1) // P) * P

    _, cj = _knot_consts()

    cvf = (np.asarray(ctrl, np.float32)
           * np.asarray(scaling, np.float32)[:, None, :])  # [E, B, U]
    proj5 = 0.5 * np.asarray(proj, np.float32)
    b1f = np.asarray(b1, np.float32)
    b1_zero = not np.any(b1f)

    g_np = ml_dtypes.bfloat16
    ar = np.arange(P)
    tiling = _tiling(C)
    NT = len(tiling)

    in_maps = []
    for e in range(E_EXP):
        idx = order[starts[e]:starts[e + 1]]
        # [NT, P, 8, TNMAX]: xT[t, p, kc, n] = x[token t0+n, kc*128+p]
        xTf = np.zeros((D_IN, C), dtype=np.float32)
        if len(idx):
            xTf[:, :len(idx)] = x[idx].T
        xT = np.zeros((len(tiling), P, 8, TNMAX), dtype=g_np)
        for ti, (t0, TN) in enumerate(tiling):
            xT[ti, :, :, :TN] = (
                xTf[:, t0:t0 + TN].reshape(8, P, TN).transpose(1, 0, 2))
        w1d = np.ascontiguousarray(
            np.asarray(W1[e], np.float32).reshape(8, P, U_DIM)
            .transpose(1, 0, 2)).astype(g_np)
        p5d = np.ascontiguousarray(
            proj5[e].reshape(4, P, U_DIM).transpose(1, 0, 2)).astype(g_np)
        b1h = np.ascontiguousarray(
            (0.5 * b1f[e]).reshape(4, P).T).astype(np.float32)
        auxn = np.zeros((P, 28, P), dtype=np.float32)
        auxd = np.zeros((P, 8, P), dtype=np.float32)
        for vc in range(4):
            for j in range(1, 8):
                auxn[ar, vc * 7 + (j - 1), ar] = \
                    cvf[e][j, vc * P:(vc + 1) * P] * cj[j]
        for j in range(8):
            auxd[ar, j, ar] = 1.0 if j == 0 else cj[j]
        a0 = np.zeros((P, 4), dtype=np.float32)
        for vc in range(4):
            a0[:, vc] = cvf[e][0, vc * P:(vc + 1) * P]
        in_maps.append({
            "xT": xT,
            "w1": w1d,
            "p5": p5d,
            "auxn": auxn.astype(g_np),
            "auxd": auxd.astype(g_np),
            "a0h": a0,
            "b1h": b1h,
            "onesd": np.ones((P, TNMAX), dtype=g_np),
        })
    return in_maps, order, starts, counts, C, b1_zero


def kernel(x, W1, b1, proj, ctrl, scaling, Wg, bg):
    from concourse.bass_utils import run_bass_kernel_spmd

    in_maps, order, starts, counts, C, b1_zero = make_in_maps(
        x, W1, b1, proj, ctrl, scaling, Wg, bg)
    nc, _ = _get_program(C, "f32r", b1_zero)

    res = run_bass_kernel_spmd(nc, in_maps, list(range(N_CORES)))

    out = np.empty((N_TOK, U_DIM), dtype=np.float32)
    for e in range(E_EXP):
        cnt = int(counts[e])
        if cnt:
            out[order[starts[e]:starts[e + 1]]] = \
                res.results[e]["outT"][:, :cnt].T
    return out


MM_MODE = "f32r"  # kept for test.py compatibility
